# revision 1
# baseline (speedup 1.0000x reference)
"""Ternary-quantized linear (CMSFlipLinear) on 8 Trainium2 NeuronCores.

Computes y = x @ W^T where W[o, i] = ternary[o, i] * scales[o*32 + i//128],
x: (4, 2048, 4096) f32, ternary: (4096, 4096), scales: (131072,) f32.

Strategy: column-parallel tensor parallelism. Each of the 8 cores owns a
512-wide slice of out_features. x is replicated (pre-transposed/tiled to
bf16 on host), ternary codes + scales are dequantized on-device into an
SBUF-resident bf16 weight, and the 8192x4096x512 matmul per core runs in
bf16 on the PE with fp32 PSUM accumulation.
"""

import sys

for _p in ("/opt/trn_rl_repo", "/opt/pypackages"):
    if _p not in sys.path:
        sys.path.append(_p)

import numpy as np
import ml_dtypes

import concourse.mybir as mybir
import concourse.tile as tile
from concourse import bacc
from concourse.bass import ts
from concourse.bass_utils import run_bass_kernel_spmd

BF16 = mybir.dt.bfloat16
F32 = mybir.dt.float32

B, S, IN, OUT = 4, 2048, 4096, 4096
R = B * S                 # 8192 rows
NCORES = 8
OSH = OUT // NCORES       # 512 out_features per core
KT = IN // 128            # 32 contraction tiles
RC = 16                   # row chunks
RCW = R // RC             # 512 rows per chunk
MSUB = RCW // 128         # 4 psum row-subtiles per chunk

_CACHE = {}


def _build():
    if "nc" in _CACHE:
        return _CACHE["nc"]

    nc = bacc.Bacc("TRN2", target_bir_lowering=False, debug=False,
                   num_devices=NCORES)

    xt = nc.dram_tensor("xt", [RC, 128, KT, RCW], BF16, kind="ExternalInput").ap()
    wt = nc.dram_tensor("wt", [KT, 128, OSH], F32, kind="ExternalInput").ap()
    sc = nc.dram_tensor("sc", [KT, 128, OSH], F32, kind="ExternalInput").ap()
    y = nc.dram_tensor("y", [RC, MSUB, 128, OSH], F32, kind="ExternalOutput").ap()

    with tile.TileContext(nc) as tc:
        with (
            tc.tile_pool(name="wpool", bufs=1) as wpool,
            tc.tile_pool(name="wstage", bufs=3) as wstage,
            tc.tile_pool(name="xpool", bufs=2) as xpool,
            tc.tile_pool(name="opool", bufs=4) as opool,
            tc.tile_pool(name="pspool", bufs=8, space="PSUM") as pspool,
        ):
            # One-time on-device dequant: wdeq[:, k, o] = ternary^T * scale
            wdeq = wpool.tile([128, KT, OSH], BF16)
            for k in range(KT):
                wtile = wstage.tile([128, OSH], F32, tag="wt")
                sctile = wstage.tile([128, OSH], F32, tag="sc")
                nc.sync.dma_start(wtile[:], wt[k])
                nc.sync.dma_start(sctile[:], sc[k])
                nc.vector.tensor_mul(out=wdeq[:, k, :], in0=wtile[:], in1=sctile[:])

            # Main loop: stream x chunks, accumulate over K into PSUM.
            for rc in range(RC):
                xsb = xpool.tile([128, KT, RCW], BF16, tag="xsb")
                nc.sync.dma_start(xsb[:], xt[rc])
                for m in range(MSUB):
                    ps = pspool.tile([128, OSH], F32, tag="ps")
                    for k in range(KT):
                        nc.tensor.matmul(
                            ps[:],
                            lhsT=xsb[:, k, ts(m, 128)],
                            rhs=wdeq[:, k, :],
                            start=(k == 0),
                            stop=(k == KT - 1),
                        )
                    osb = opool.tile([128, OSH], F32, tag="osb")
                    nc.vector.tensor_copy(out=osb[:], in_=ps[:])
                    nc.sync.dma_start(y[rc, m], osb[:])

    nc.compile()
    _CACHE["nc"] = nc
    return nc


def _prep_inputs(x, ternary, scales):
    x = np.asarray(x, dtype=np.float32).reshape(R, IN)
    ternary = np.asarray(ternary)
    scales = np.asarray(scales, dtype=np.float32)

    # x -> bf16, tiled [rc, p, k, r'] with p the contraction partition
    xb = x.astype(ml_dtypes.bfloat16)
    xt5 = np.ascontiguousarray(
        xb.reshape(RC, RCW, KT, 128).transpose(0, 3, 2, 1)
    )

    sc_full = scales.reshape(OUT, KT)  # [o, k] with k = i // 128

    in_maps = []
    for c in range(NCORES):
        tern_c = ternary[c * OSH:(c + 1) * OSH, :].astype(np.float32)
        wt_c = np.ascontiguousarray(tern_c.T).reshape(KT, 128, OSH)
        sc_c = np.ascontiguousarray(
            np.broadcast_to(
                sc_full[c * OSH:(c + 1) * OSH, :].T[:, None, :], (KT, 128, OSH)
            )
        )
        in_maps.append({"xt": xt5, "wt": wt_c, "sc": sc_c})
    return in_maps


def _run(in_maps, trace=False, tmpdir=None):
    nc = _build()
    return run_bass_kernel_spmd(
        nc, in_maps, core_ids=list(range(NCORES)), trace=trace, tmpdir=tmpdir
    )


def kernel(x, ternary, scales):
    in_maps = _prep_inputs(x, ternary, scales)
    res = _run(in_maps)
    out = np.empty((R, OUT), dtype=np.float32)
    for c in range(NCORES):
        out[:, c * OSH:(c + 1) * OSH] = res.results[c]["y"].reshape(R, OSH)
    return out.reshape(B, S, OUT)


# revision 4
# speedup vs baseline: 1.0310x; 1.0310x over previous
"""Ternary-quantized linear (CMSFlipLinear) on 8 Trainium2 NeuronCores.

Computes y = x @ W^T where W[o, i] = ternary[o, i] * scales[o*32 + i//128],
x: (4, 2048, 4096) f32, ternary: (4096, 4096), scales: (131072,) f32.

Strategy: column-parallel tensor parallelism. Each of the 8 cores owns a
512-wide slice of out_features. x is replicated (pre-transposed/tiled to
bf16 on host), ternary codes + scales are dequantized on-device into an
SBUF-resident bf16 weight, and the 8192x4096x512 matmul per core runs in
bf16 on the PE with fp32 PSUM accumulation.
"""

import sys

for _p in ("/opt/trn_rl_repo", "/opt/pypackages"):
    if _p not in sys.path:
        sys.path.append(_p)

import numpy as np
import ml_dtypes

import concourse.mybir as mybir
import concourse.tile as tile
from concourse import bacc
from concourse.bass import ts
from concourse.bass_utils import run_bass_kernel_spmd

BF16 = mybir.dt.bfloat16
F32 = mybir.dt.float32

B, S, IN, OUT = 4, 2048, 4096, 4096
R = B * S                 # 8192 rows
NCORES = 8
OSH = OUT // NCORES       # 512 out_features per core
KT = IN // 128            # 32 contraction tiles
RC = 16                   # row chunks
RCW = R // RC             # 512 rows per chunk
MSUB = RCW // 128         # 4 psum row-subtiles per chunk

_CACHE = {}


def _build():
    if "nc" in _CACHE:
        return _CACHE["nc"]

    nc = bacc.Bacc("TRN2", target_bir_lowering=False, debug=False,
                   num_devices=NCORES)

    xt = nc.dram_tensor("xt", [RC, 128, KT, RCW], BF16, kind="ExternalInput").ap()
    wt = nc.dram_tensor("wt", [KT, 128, OSH], BF16, kind="ExternalInput").ap()
    sc = nc.dram_tensor("sc", [KT, 128, OSH], BF16, kind="ExternalInput").ap()
    y = nc.dram_tensor("y", [RC, MSUB, 128, OSH], F32, kind="ExternalOutput").ap()

    with tile.TileContext(nc) as tc:
        with (
            tc.tile_pool(name="wpool", bufs=1) as wpool,
            tc.tile_pool(name="wstage", bufs=4) as wstage,
            tc.tile_pool(name="xpool", bufs=2) as xpool,
            tc.tile_pool(name="opool", bufs=4) as opool,
            tc.tile_pool(name="pspool", bufs=8, space="PSUM") as pspool,
        ):
            wdeq = wpool.tile([128, KT, OSH], BF16)
            xsb0 = xpool.tile([128, KT, RCW], BF16, tag="xsb")

            # Startup: interleave per-k weight dequant with per-k slices of
            # the first x chunk, so matmuls can begin after the first k-tile.
            for k in range(KT):
                wtile = wstage.tile([128, OSH], BF16, tag="wt")
                sctile = wstage.tile([128, OSH], BF16, tag="sc")
                nc.sync.dma_start(wtile[:], wt[k])
                nc.sync.dma_start(sctile[:], sc[k])
                nc.vector.tensor_mul(out=wdeq[:, k, :], in0=wtile[:], in1=sctile[:])
                nc.sync.dma_start(xsb0[:, k, :], xt[0, :, k, :])

            # Main loop. k-outer / m-inner: MM(k) only depends on wdeq[k] and
            # xsb[:, k, :], so the PE starts as soon as the first tiles land.
            for rc in range(RC):
                if rc == 0:
                    xsb = xsb0
                else:
                    xsb = xpool.tile([128, KT, RCW], BF16, tag="xsb")
                    nc.sync.dma_start(xsb[:], xt[rc])
                pss = [
                    pspool.tile([128, OSH], F32, tag="ps", name=f"ps_{rc}_{m}")
                    for m in range(MSUB)
                ]
                for k in range(KT):
                    for m in range(MSUB):
                        nc.tensor.matmul(
                            pss[m][:],
                            lhsT=xsb[:, k, ts(m, 128)],
                            rhs=wdeq[:, k, :],
                            start=(k == 0),
                            stop=(k == KT - 1),
                        )
                for m in range(MSUB):
                    osb = opool.tile([128, OSH], F32, tag="osb")
                    nc.vector.tensor_copy(out=osb[:], in_=pss[m][:])
                    nc.sync.dma_start(y[rc, m], osb[:])

    nc.compile()
    _CACHE["nc"] = nc
    return nc


def _prep_inputs(x, ternary, scales):
    x = np.asarray(x, dtype=np.float32).reshape(R, IN)
    ternary = np.asarray(ternary)
    scales = np.asarray(scales, dtype=np.float32)

    # x -> bf16, tiled [rc, p, k, r'] with p the contraction partition
    xb = x.astype(ml_dtypes.bfloat16)
    xt5 = np.ascontiguousarray(
        xb.reshape(RC, RCW, KT, 128).transpose(0, 3, 2, 1)
    )

    sc_full = scales.reshape(OUT, KT)  # [o, k] with k = i // 128

    in_maps = []
    for c in range(NCORES):
        tern_c = ternary[c * OSH:(c + 1) * OSH, :].astype(ml_dtypes.bfloat16)
        wt_c = np.ascontiguousarray(tern_c.T).reshape(KT, 128, OSH)
        sc_c = np.ascontiguousarray(
            np.broadcast_to(
                sc_full[c * OSH:(c + 1) * OSH, :]
                .astype(ml_dtypes.bfloat16)
                .T[:, None, :],
                (KT, 128, OSH),
            )
        )
        in_maps.append({"xt": xt5, "wt": wt_c, "sc": sc_c})
    return in_maps


def _run(in_maps, trace=False, tmpdir=None):
    nc = _build()
    return run_bass_kernel_spmd(
        nc, in_maps, core_ids=list(range(NCORES)), trace=trace, tmpdir=tmpdir
    )


def kernel(x, ternary, scales):
    in_maps = _prep_inputs(x, ternary, scales)
    res = _run(in_maps)
    out = np.empty((R, OUT), dtype=np.float32)
    for c in range(NCORES):
        out[:, c * OSH:(c + 1) * OSH] = res.results[c]["y"].reshape(R, OSH)
    return out.reshape(B, S, OUT)


# revision 8
# speedup vs baseline: 1.0945x; 1.0616x over previous
"""Ternary-quantized linear (CMSFlipLinear) on 8 Trainium2 NeuronCores.

Computes y = x @ W^T where W[o, i] = ternary[o, i] * scales[o*32 + i//128],
x: (4, 2048, 4096) f32, ternary: (4096, 4096), scales: (131072,) f32.

Strategy: column-parallel tensor parallelism. Each of the 8 cores owns a
512-wide slice of out_features. x is replicated (pre-transposed/tiled to
bf16 on host), ternary codes + scales are dequantized on-device into an
SBUF-resident bf16 weight, and the 8192x4096x512 matmul per core runs in
bf16 on the PE with fp32 PSUM accumulation.
"""

import sys

for _p in ("/opt/trn_rl_repo", "/opt/pypackages"):
    if _p not in sys.path:
        sys.path.append(_p)

import numpy as np
import ml_dtypes

import concourse.bass as bass
import concourse.mybir as mybir
import concourse.tile as tile
from concourse import bacc
from concourse.bass import ts
from concourse.bass_utils import run_bass_kernel_spmd

BF16 = mybir.dt.bfloat16
F32 = mybir.dt.float32

B, S, IN, OUT = 4, 2048, 4096, 4096
R = B * S                 # 8192 rows
NCORES = 8
OSH = OUT // NCORES       # 512 out_features per core
KT = IN // 128            # 32 contraction tiles
RC = 16                   # row chunks
RCW = R // RC             # 512 rows per chunk
MSUB = RCW // 128         # 4 psum row-subtiles per chunk

_CACHE = {}


def _build():
    if "nc" in _CACHE:
        return _CACHE["nc"]

    nc = bacc.Bacc("TRN2", target_bir_lowering=False, debug=False,
                   num_devices=NCORES)

    KG = 4                 # k-tiles per weight-prep group
    NG = KT // KG          # 8 groups

    xt = nc.dram_tensor("xt", [RC, 128, KT, RCW], BF16, kind="ExternalInput").ap()
    wt = nc.dram_tensor("wt", [NG, 128, KG, OSH], BF16, kind="ExternalInput").ap()
    sc = nc.dram_tensor("sc", [KT * OSH], BF16, kind="ExternalInput").ap()
    y = nc.dram_tensor("y", [RC, MSUB, 128, OSH], F32, kind="ExternalOutput").ap()

    with tile.TileContext(nc) as tc:
        with (
            tc.tile_pool(name="wpool", bufs=1) as wpool,
            tc.tile_pool(name="wstage", bufs=3) as wstage,
            tc.tile_pool(name="xpool", bufs=2) as xpool,
            tc.tile_pool(name="opool", bufs=4) as opool,
            tc.tile_pool(name="pspool", bufs=8, space="PSUM") as pspool,
        ):
            wdeq = wpool.tile([128, KT, OSH], BF16)
            scb = wpool.tile([128, KT, OSH], BF16)
            xsb0 = xpool.tile([128, KT, RCW], BF16, tag="xsb")

            # Startup: per-group weight dequant interleaved with slices of the
            # first x chunk, spread across three DMA-issue queues (scalar /
            # gpsimd / sync) so issue serialization can't starve the PE.
            for g in range(NG):
                wtile = wstage.tile([128, KG, OSH], BF16, tag="wt")
                nc.scalar.dma_start(wtile[:], wt[g])
                sc_src = sc[g * KG * OSH:(g + 1) * KG * OSH]
                sc_bcast = bass.AP(
                    tensor=sc_src.tensor,
                    offset=sc_src.offset,
                    ap=[[0, 128], [1, KG * OSH]],
                )
                nc.gpsimd.dma_start(out=scb[:, ts(g, KG), :], in_=sc_bcast)
                nc.vector.tensor_mul(
                    out=wdeq[:, ts(g, KG), :],
                    in0=wtile[:],
                    in1=scb[:, ts(g, KG), :],
                )
                nc.sync.dma_start(xsb0[:, ts(g, KG), :], xt[0, :, ts(g, KG), :])

            # Prefetch the second x chunk before entering the matmul stream.
            xsb1 = xpool.tile([128, KT, RCW], BF16, tag="xsb")
            nc.sync.dma_start(xsb1[:], xt[1])

            # Main loop. k-outer / m-inner: MM(k) only depends on wdeq[k] and
            # xsb[:, k, :], so the PE starts as soon as the first tiles land.
            # The last chunk runs m-outer so psum eviction overlaps the tail.
            for rc in range(RC):
                if rc == 0:
                    xsb = xsb0
                elif rc == 1:
                    xsb = xsb1
                else:
                    xsb = xpool.tile([128, KT, RCW], BF16, tag="xsb")
                    nc.sync.dma_start(xsb[:], xt[rc])
                pss = [
                    pspool.tile([128, OSH], F32, tag="ps", name=f"ps_{rc}_{m}")
                    for m in range(MSUB)
                ]
                last = rc == RC - 1
                loop = (
                    [(k, m) for m in range(MSUB) for k in range(KT)]
                    if last
                    else [(k, m) for k in range(KT) for m in range(MSUB)]
                )
                for k, m in loop:
                    nc.tensor.matmul(
                        pss[m][:],
                        lhsT=xsb[:, k, ts(m, 128)],
                        rhs=wdeq[:, k, :],
                        start=(k == 0),
                        stop=(k == KT - 1),
                    )
                    if last and k == KT - 1:
                        osb = opool.tile(
                            [128, OSH], F32, tag="osb", name=f"osb_{rc}_{m}"
                        )
                        nc.vector.tensor_copy(out=osb[:], in_=pss[m][:])
                        nc.sync.dma_start(y[rc, m], osb[:])
                if not last:
                    for m in range(MSUB):
                        osb = opool.tile(
                            [128, OSH], F32, tag="osb", name=f"osb_{rc}_{m}"
                        )
                        nc.vector.tensor_copy(out=osb[:], in_=pss[m][:])
                        nc.sync.dma_start(y[rc, m], osb[:])

    nc.compile()
    _CACHE["nc"] = nc
    return nc


def _prep_inputs(x, ternary, scales):
    x = np.asarray(x, dtype=np.float32).reshape(R, IN)
    ternary = np.asarray(ternary)
    scales = np.asarray(scales, dtype=np.float32)

    # x -> bf16, tiled [rc, p, k, r'] with p the contraction partition
    xb = x.astype(ml_dtypes.bfloat16)
    xt5 = np.ascontiguousarray(
        xb.reshape(RC, RCW, KT, 128).transpose(0, 3, 2, 1)
    )

    sc_full = scales.reshape(OUT, KT)  # [o, k] with k = i // 128

    in_maps = []
    for c in range(NCORES):
        tern_c = ternary[c * OSH:(c + 1) * OSH, :].astype(ml_dtypes.bfloat16)
        wt_c = np.ascontiguousarray(
            np.ascontiguousarray(tern_c.T)
            .reshape(8, 4, 128, OSH)
            .transpose(0, 2, 1, 3)
        )
        sc_c = np.ascontiguousarray(
            sc_full[c * OSH:(c + 1) * OSH, :].astype(ml_dtypes.bfloat16).T
        ).reshape(KT * OSH)
        in_maps.append({"xt": xt5, "wt": wt_c, "sc": sc_c})
    return in_maps


def _run(in_maps, trace=False, tmpdir=None):
    nc = _build()
    return run_bass_kernel_spmd(
        nc, in_maps, core_ids=list(range(NCORES)), trace=trace, tmpdir=tmpdir
    )


def kernel(x, ternary, scales):
    in_maps = _prep_inputs(x, ternary, scales)
    res = _run(in_maps)
    out = np.empty((R, OUT), dtype=np.float32)
    for c in range(NCORES):
        out[:, c * OSH:(c + 1) * OSH] = res.results[c]["y"].reshape(R, OSH)
    return out.reshape(B, S, OUT)


# revision 11
# speedup vs baseline: 1.0956x; 1.0010x over previous
"""Ternary-quantized linear (CMSFlipLinear) on 8 Trainium2 NeuronCores.

Computes y = x @ W^T where W[o, i] = ternary[o, i] * scales[o*32 + i//128],
x: (4, 2048, 4096) f32, ternary: (4096, 4096), scales: (131072,) f32.

Strategy: column-parallel tensor parallelism. Each of the 8 cores owns a
512-wide slice of out_features. x is replicated (pre-transposed/tiled to
bf16 on host), ternary codes + scales are dequantized on-device into an
SBUF-resident bf16 weight, and the 8192x4096x512 matmul per core runs in
bf16 on the PE with fp32 PSUM accumulation.
"""

import sys

for _p in ("/opt/trn_rl_repo", "/opt/pypackages"):
    if _p not in sys.path:
        sys.path.append(_p)

import numpy as np
import ml_dtypes

import concourse.bass as bass
import concourse.mybir as mybir
import concourse.tile as tile
from concourse import bacc
from concourse.bass import ts
from concourse.bass_utils import run_bass_kernel_spmd

BF16 = mybir.dt.bfloat16
F32 = mybir.dt.float32

B, S, IN, OUT = 4, 2048, 4096, 4096
R = B * S                 # 8192 rows
NCORES = 8
OSH = OUT // NCORES       # 512 out_features per core
KT = IN // 128            # 32 contraction tiles
RC = 16                   # row chunks
RCW = R // RC             # 512 rows per chunk
MSUB = RCW // 128         # 4 psum row-subtiles per chunk

_CACHE = {}


def _build():
    if "nc" in _CACHE:
        return _CACHE["nc"]

    nc = bacc.Bacc("TRN2", target_bir_lowering=False, debug=False,
                   num_devices=NCORES)

    KG = 4                 # k-tiles per weight-prep group
    NG = KT // KG          # 8 groups

    I8 = mybir.dt.int8
    xt = nc.dram_tensor("xt", [RC, 128, KT, RCW], BF16, kind="ExternalInput").ap()
    wt = nc.dram_tensor("wt", [NG, 128, KG, OSH], I8, kind="ExternalInput").ap()
    sc = nc.dram_tensor("sc", [NG, 128, KG, OSH], BF16, kind="ExternalInput").ap()
    y = nc.dram_tensor("y", [RC, MSUB, 128, OSH], F32, kind="ExternalOutput").ap()

    with tile.TileContext(nc) as tc:
        with (
            tc.tile_pool(name="wpool", bufs=1) as wpool,
            tc.tile_pool(name="wstage", bufs=3) as wstage,
            tc.tile_pool(name="xpool", bufs=2) as xpool,
            tc.tile_pool(name="opool", bufs=4) as opool,
            tc.tile_pool(name="pspool", bufs=8, space="PSUM") as pspool,
        ):
            wdeq = wpool.tile([128, KT, OSH], BF16)
            xsb0 = xpool.tile([128, KT, RCW], BF16, tag="xsb")

            # Startup: per-group weight dequant interleaved with slices of the
            # first x chunk, spread across three DMA-issue queues. Ternary
            # codes travel as int8 and are cast to bf16 by the SWDGE DMA.
            for g in range(NG):
                wtile = wstage.tile([128, KG, OSH], BF16, tag="wt")
                sctile = wstage.tile([128, KG, OSH], BF16, tag="sc")
                nc.gpsimd.dma_start(out=wtile[:], in_=wt[g])
                nc.scalar.dma_start(sctile[:], sc[g])
                nc.vector.tensor_mul(
                    out=wdeq[:, ts(g, KG), :],
                    in0=wtile[:],
                    in1=sctile[:],
                )
                nc.sync.dma_start(xsb0[:, ts(g, KG), :], xt[0, :, ts(g, KG), :])

            # Prefetch the second x chunk before entering the matmul stream.
            xsb1 = xpool.tile([128, KT, RCW], BF16, tag="xsb")
            nc.sync.dma_start(xsb1[:], xt[1])

            # Main loop. k-outer / m-inner: MM(k) only depends on wdeq[k] and
            # xsb[:, k, :], so the PE starts as soon as the first tiles land.
            # The last chunk runs m-outer so psum eviction overlaps the tail.
            for rc in range(RC):
                if rc == 0:
                    xsb = xsb0
                elif rc == 1:
                    xsb = xsb1
                else:
                    xsb = xpool.tile([128, KT, RCW], BF16, tag="xsb")
                    nc.sync.dma_start(xsb[:], xt[rc])
                pss = [
                    pspool.tile([128, OSH], F32, tag="ps", name=f"ps_{rc}_{m}")
                    for m in range(MSUB)
                ]
                last = rc == RC - 1
                loop = (
                    [(k, m) for m in range(MSUB) for k in range(KT)]
                    if last
                    else [(k, m) for k in range(KT) for m in range(MSUB)]
                )
                for k, m in loop:
                    nc.tensor.matmul(
                        pss[m][:],
                        lhsT=xsb[:, k, ts(m, 128)],
                        rhs=wdeq[:, k, :],
                        start=(k == 0),
                        stop=(k == KT - 1),
                    )
                    if last and k == KT - 1:
                        osb = opool.tile(
                            [128, OSH], F32, tag="osb", name=f"osb_{rc}_{m}"
                        )
                        nc.vector.tensor_copy(out=osb[:], in_=pss[m][:])
                        nc.sync.dma_start(y[rc, m], osb[:])
                if not last:
                    for m in range(MSUB):
                        osb = opool.tile(
                            [128, OSH], F32, tag="osb", name=f"osb_{rc}_{m}"
                        )
                        nc.vector.tensor_copy(out=osb[:], in_=pss[m][:])
                        nc.sync.dma_start(y[rc, m], osb[:])

    nc.compile()
    _CACHE["nc"] = nc
    return nc


def _prep_inputs(x, ternary, scales):
    x = np.asarray(x, dtype=np.float32).reshape(R, IN)
    ternary = np.asarray(ternary)
    scales = np.asarray(scales, dtype=np.float32)

    # x -> bf16, tiled [rc, p, k, r'] with p the contraction partition
    xb = x.astype(ml_dtypes.bfloat16)
    xt5 = np.ascontiguousarray(
        xb.reshape(RC, RCW, KT, 128).transpose(0, 3, 2, 1)
    )

    sc_full = scales.reshape(OUT, KT)  # [o, k] with k = i // 128

    in_maps = []
    for c in range(NCORES):
        tern_c = ternary[c * OSH:(c + 1) * OSH, :].astype(np.int8)
        wt_c = np.ascontiguousarray(
            np.ascontiguousarray(tern_c.T)
            .reshape(8, 4, 128, OSH)
            .transpose(0, 2, 1, 3)
        )
        sc_kt = np.ascontiguousarray(
            sc_full[c * OSH:(c + 1) * OSH, :].astype(ml_dtypes.bfloat16).T
        )  # [KT, OSH]
        sc_c = np.ascontiguousarray(
            np.broadcast_to(
                sc_kt.reshape(8, 4, 1, OSH), (8, 4, 128, OSH)
            ).transpose(0, 2, 1, 3)
        )
        in_maps.append({"xt": xt5, "wt": wt_c, "sc": sc_c})
    return in_maps


def _run(in_maps, trace=False, tmpdir=None):
    nc = _build()
    return run_bass_kernel_spmd(
        nc, in_maps, core_ids=list(range(NCORES)), trace=trace, tmpdir=tmpdir
    )


def kernel(x, ternary, scales):
    in_maps = _prep_inputs(x, ternary, scales)
    res = _run(in_maps)
    out = np.empty((R, OUT), dtype=np.float32)
    for c in range(NCORES):
        out[:, c * OSH:(c + 1) * OSH] = res.results[c]["y"].reshape(R, OSH)
    return out.reshape(B, S, OUT)


# revision 12
# speedup vs baseline: 1.0978x; 1.0020x over previous
"""Ternary-quantized linear (CMSFlipLinear) on 8 Trainium2 NeuronCores.

Computes y = x @ W^T where W[o, i] = ternary[o, i] * scales[o*32 + i//128],
x: (4, 2048, 4096) f32, ternary: (4096, 4096), scales: (131072,) f32.

Strategy: column-parallel tensor parallelism. Each of the 8 cores owns a
512-wide slice of out_features. x is replicated (pre-transposed/tiled to
bf16 on host), ternary codes + scales are dequantized on-device into an
SBUF-resident bf16 weight, and the 8192x4096x512 matmul per core runs in
bf16 on the PE with fp32 PSUM accumulation.
"""

import sys

for _p in ("/opt/trn_rl_repo", "/opt/pypackages"):
    if _p not in sys.path:
        sys.path.append(_p)

import numpy as np
import ml_dtypes

import concourse.bass as bass
import concourse.mybir as mybir
import concourse.tile as tile
from concourse import bacc
from concourse.bass import ts
from concourse.bass_utils import run_bass_kernel_spmd

BF16 = mybir.dt.bfloat16
F32 = mybir.dt.float32

B, S, IN, OUT = 4, 2048, 4096, 4096
R = B * S                 # 8192 rows
NCORES = 8
OSH = OUT // NCORES       # 512 out_features per core
KT = IN // 128            # 32 contraction tiles
RC = 16                   # row chunks
RCW = R // RC             # 512 rows per chunk
MSUB = RCW // 128         # 4 psum row-subtiles per chunk

_CACHE = {}


def _build():
    if "nc" in _CACHE:
        return _CACHE["nc"]

    nc = bacc.Bacc("TRN2", target_bir_lowering=False, debug=False,
                   num_devices=NCORES)

    KG = 4                 # k-tiles per weight-prep group
    NG = KT // KG          # 8 groups

    I8 = mybir.dt.int8
    xt = nc.dram_tensor("xt", [RC, 128, KT, RCW], BF16, kind="ExternalInput").ap()
    wt = nc.dram_tensor("wt", [NG, 128, KG, OSH], I8, kind="ExternalInput").ap()
    sc = nc.dram_tensor("sc", [NG, 128, KG, OSH], BF16, kind="ExternalInput").ap()
    y = nc.dram_tensor("y", [RC, MSUB, 128, OSH], F32, kind="ExternalOutput").ap()

    with tile.TileContext(nc) as tc:
        with (
            tc.tile_pool(name="wpool", bufs=1) as wpool,
            tc.tile_pool(name="wstage", bufs=3) as wstage,
            tc.tile_pool(name="xpool", bufs=2) as xpool,
            tc.tile_pool(name="opool", bufs=4) as opool,
            tc.tile_pool(name="pspool", bufs=8, space="PSUM") as pspool,
        ):
            wdeq = wpool.tile([128, KT, OSH], BF16)
            xsb0 = xpool.tile([128, KT, RCW], BF16, tag="xsb")

            # Startup: per-group weight dequant interleaved with slices of the
            # first x chunk, spread across three DMA-issue queues. Ternary
            # codes travel as int8 and are cast to bf16 by the SWDGE DMA.
            for g in range(NG):
                wtile = wstage.tile([128, KG, OSH], I8, tag="wt")
                sctile = wstage.tile([128, KG, OSH], BF16, tag="sc")
                nc.scalar.dma_start(wtile[:], wt[g])
                nc.scalar.dma_start(sctile[:], sc[g])
                nc.vector.tensor_mul(
                    out=wdeq[:, ts(g, KG), :],
                    in0=wtile[:],
                    in1=sctile[:],
                )
                nc.sync.dma_start(xsb0[:, ts(g, KG), :], xt[0, :, ts(g, KG), :])

            # Prefetch the second x chunk before entering the matmul stream.
            xsb1 = xpool.tile([128, KT, RCW], BF16, tag="xsb")
            nc.sync.dma_start(xsb1[:], xt[1])

            # Main loop. k-outer / m-inner: MM(k) only depends on wdeq[k] and
            # xsb[:, k, :], so the PE starts as soon as the first tiles land.
            # The last chunk runs m-outer so psum eviction overlaps the tail.
            for rc in range(RC):
                if rc == 0:
                    xsb = xsb0
                elif rc == 1:
                    xsb = xsb1
                else:
                    xsb = xpool.tile([128, KT, RCW], BF16, tag="xsb")
                    nc.sync.dma_start(xsb[:], xt[rc])
                pss = [
                    pspool.tile([128, OSH], F32, tag="ps", name=f"ps_{rc}_{m}")
                    for m in range(MSUB)
                ]
                last = rc == RC - 1
                loop = (
                    [(k, m) for m in range(MSUB) for k in range(KT)]
                    if last
                    else [(k, m) for k in range(KT) for m in range(MSUB)]
                )
                for k, m in loop:
                    nc.tensor.matmul(
                        pss[m][:],
                        lhsT=xsb[:, k, ts(m, 128)],
                        rhs=wdeq[:, k, :],
                        start=(k == 0),
                        stop=(k == KT - 1),
                    )
                    if last and k == KT - 1:
                        osb = opool.tile(
                            [128, OSH], F32, tag="osb", name=f"osb_{rc}_{m}"
                        )
                        nc.vector.tensor_copy(out=osb[:], in_=pss[m][:])
                        nc.sync.dma_start(y[rc, m], osb[:])
                if not last:
                    for m in range(MSUB):
                        osb = opool.tile(
                            [128, OSH], F32, tag="osb", name=f"osb_{rc}_{m}"
                        )
                        nc.vector.tensor_copy(out=osb[:], in_=pss[m][:])
                        nc.sync.dma_start(y[rc, m], osb[:])

    nc.compile()
    _CACHE["nc"] = nc
    return nc


def _prep_inputs(x, ternary, scales):
    x = np.asarray(x, dtype=np.float32).reshape(R, IN)
    ternary = np.asarray(ternary)
    scales = np.asarray(scales, dtype=np.float32)

    # x -> bf16, tiled [rc, p, k, r'] with p the contraction partition
    xb = x.astype(ml_dtypes.bfloat16)
    xt5 = np.ascontiguousarray(
        xb.reshape(RC, RCW, KT, 128).transpose(0, 3, 2, 1)
    )

    sc_full = scales.reshape(OUT, KT)  # [o, k] with k = i // 128

    in_maps = []
    for c in range(NCORES):
        tern_c = ternary[c * OSH:(c + 1) * OSH, :].astype(np.int8)
        wt_c = np.ascontiguousarray(
            np.ascontiguousarray(tern_c.T)
            .reshape(8, 4, 128, OSH)
            .transpose(0, 2, 1, 3)
        )
        sc_kt = np.ascontiguousarray(
            sc_full[c * OSH:(c + 1) * OSH, :].astype(ml_dtypes.bfloat16).T
        )  # [KT, OSH]
        sc_c = np.ascontiguousarray(
            np.broadcast_to(
                sc_kt.reshape(8, 4, 1, OSH), (8, 4, 128, OSH)
            ).transpose(0, 2, 1, 3)
        )
        in_maps.append({"xt": xt5, "wt": wt_c, "sc": sc_c})
    return in_maps


def _run(in_maps, trace=False, tmpdir=None):
    nc = _build()
    return run_bass_kernel_spmd(
        nc, in_maps, core_ids=list(range(NCORES)), trace=trace, tmpdir=tmpdir
    )


def kernel(x, ternary, scales):
    in_maps = _prep_inputs(x, ternary, scales)
    res = _run(in_maps)
    out = np.empty((R, OUT), dtype=np.float32)
    for c in range(NCORES):
        out[:, c * OSH:(c + 1) * OSH] = res.results[c]["y"].reshape(R, OSH)
    return out.reshape(B, S, OUT)
